# revision 17
# baseline (speedup 1.0000x reference)
"""Multi-head causal attention (B=4, T=2048, DM=1024, H=16, DK=DV=64) on 8 TRN2
NeuronCores.

Sharding: tensor-parallel over heads — core c owns heads {2c, 2c+1}. Each core:
  1. projects full-sequence Q^T/K^T/V^T for its 2 heads from a pre-transposed
     x^T (host supplies x^T; pure layout prep, no FLOPs),
  2. runs causal attention per (batch, head) in S^T = K Q^T layout with
     block-skipping of the fully-masked triangle; softmax denominators ride
     along as a ones-column appended to V (row 64 of the PV accumulator);
     normalization is deferred to a per-column reciprocal broadcast built with
     a K=1 PE matmul,
  3. row-shards W_o: out_partial = concat(head_out).T @ Wo[128c:128c+128] + bo/8.
Host sums the 8 partials (the W_o all-reduce done on host).
"""

import sys

for _p in ("/opt/trn_rl_repo",):
    if _p not in sys.path:
        sys.path.insert(0, _p)

import numpy as np

# ---- problem constants (hardcoded per harness contract) ----
B, T, DM = 4, 2048, 1024
H, DK = 16, 64
NCORES = 8
HL = 2                      # heads per core
SD = HL * DK                # 128: partition width of per-core head-stacked tiles
BT = B * T

# tiling
TB = 512                    # projection t-block (moving N)
PW = 1024                   # attention q "pair" width (PSUM S tile free size)
CH = 512                    # PSUM chunk / matmul moving width
ND = DM // 128              # contraction k-tiles for projections
NT = T // 128               # s-tiles per batch
NPAIR = T // PW
TBB = T // TB               # projection t-blocks per batch
SCALE = DK ** -0.5

_CACHE = {}


def _build(mode="f32"):
    import concourse.bass as bass
    import concourse.tile as tile
    from concourse import bacc, mybir

    f32 = mybir.dt.float32
    mdt = {"f32": f32, "f32r": mybir.dt.float32r,
           "bf16": mybir.dt.bfloat16}[mode]
    # denominator path stays exact: bf16 rowsums would inject ~4e-3 error
    ddt = f32 if mode == "bf16" else mdt
    ts = bass.ts

    def rc(ap):  # matmul operands are natively typed now
        return ap

    def dma_in(out, in_):  # SWDGE casts on the fly; HWDGE needs equal dtypes
        if out.dtype != in_.dtype:
            return nc.gpsimd.dma_start(out=out, in_=in_)
        return nc.sync.dma_start(out=out, in_=in_)

    nc = bacc.Bacc("TRN2", target_bir_lowering=False, debug=False,
                   num_devices=NCORES)

    xT = nc.dram_tensor("xT", [DM, BT], f32, kind="ExternalInput").ap()
    wq2 = nc.dram_tensor("wq2", [DM, SD], f32, kind="ExternalInput").ap()
    wk2 = nc.dram_tensor("wk2", [DM, SD], f32, kind="ExternalInput").ap()
    wv2 = nc.dram_tensor("wv2", [DM, SD], f32, kind="ExternalInput").ap()
    wo_my = nc.dram_tensor("wo_my", [SD, DM], f32, kind="ExternalInput").ap()
    bo8 = nc.dram_tensor("bo8", [1, DM], f32, kind="ExternalInput").ap()
    outp = nc.dram_tensor("out_part", [BT, DM], f32, kind="ExternalOutput").ap()

    # additive causal mask in S^T layout: -1e30 where q < s (strict lower)
    tri = ((1.0 - np.triu(np.ones((128, 128)))) * -1e30).astype(np.float32)
    # broadcast row lives at partition 64 so its base partition matches the
    # accumulator denominator row it multiplies (matmul requires equal bases)
    ones2 = np.zeros((65, 64), np.float32)
    ones2[64, :] = 1.0
    ident = np.concatenate([np.eye(64, dtype=np.float32)] * 2, 0)  # [128, 64]
    vones = np.ones((128, NT, 1), np.float32)
    tri_h = nc.inline_tensor(tri, name="tri_const")
    vones_h = nc.inline_tensor(vones, name="vones_const")
    ones2_h = nc.inline_tensor(ones2, name="ones2_const")
    id_h = nc.inline_tensor(ident, name="id_const")

    with tile.TileContext(nc) as tc:
        with (
            tc.tile_pool(name="singles", bufs=1) as singles,
            tc.tile_pool(name="stream", bufs=2) as stream,
            tc.tile_pool(name="seq", bufs=2) as seq,
            tc.tile_pool(name="att", bufs=3) as att,
            tc.tile_pool(name="small", bufs=4) as small,
            tc.tile_pool(name="pmm", bufs=2, space="PSUM") as pmm,
            tc.tile_pool(name="pacc", bufs=4, space="PSUM") as pacc,
        ):
            # ---- constants into SBUF ----
            tri_sb = singles.tile([128, 128], f32, tag="tri")
            nc.sync.dma_start(out=tri_sb, in_=tri_h.ap())
            ones2_sb = singles.tile([65, 64], ddt, tag="ones2")
            dma_in(ones2_sb, ones2_h.ap())
            id_sb = singles.tile([128, 64], f32, tag="ident")
            nc.sync.dma_start(out=id_sb, in_=id_h.ap())

            bo_sb = singles.tile([128, DM], f32, tag="bo")
            nc.sync.dma_start(
                out=bo_sb,
                in_=bass.AP(tensor=bo8.tensor, offset=0, ap=[[0, 128], [1, DM]]),
            )
            wo_sb = singles.tile([128, DM], mdt, tag="wo")
            dma_in(wo_sb, wo_my)

            w_sb = {}
            for name, src in (("q", wq2), ("k", wk2), ("v", wv2)):
                w_sb[name] = singles.tile([128, ND, SD], mdt, tag=f"w{name}", name=f"w{name}_sb")
                dma_in(
                    w_sb[name],
                    src.rearrange("(a p) m -> p a m", p=128),
                )

            xT_r = xT.rearrange("(a p) t -> p a t", p=128)

            for b in range(B):
                # ================= phase A: projections for batch b ========
                qt = seq.tile([128, T], mdt, tag="qt")
                kt = seq.tile([128, T], mdt, tag="kt")
                vt = seq.tile([128, T], f32, tag="vt")
                for i in range(TBB):
                    xts = stream.tile([128, ND, TB], mdt, tag="xts")
                    dma_in(
                        xts,
                        xT_r[:, :, b * T + i * TB: b * T + (i + 1) * TB],
                    )
                    for name, dst in (("q", qt), ("k", kt), ("v", vt)):
                        pj = pmm.tile([128, TB], f32, tag="s")
                        for a in range(ND):
                            nc.tensor.matmul(
                                pj,
                                rc(w_sb[name][:, a, :]),
                                rc(xts[:, a, :]),
                                start=(a == 0),
                                stop=(a == ND - 1),
                            )
                        nc.vector.tensor_copy(dst[:, ts(i, TB)], pj)

                # V into [t, v] layout with ones columns: per s-tile j the
                # block is [h0 v(64) | 1 | h1 v(64) | 1] -> 130 cols
                vsb = seq.tile([128, NT * 130], mdt, tag="vsb")
                vsb3 = vsb.rearrange("p (n c) -> p n c", c=130)
                dma_in(vsb3[:, :, 64:65], vones_h.ap())
                dma_in(vsb3[:, :, 129:130], vones_h.ap())
                for j in range(NT):
                    for h in (0, 1):
                        ptr = pmm.tile([128, 64], f32, tag="s")
                        nc.tensor.transpose(
                            ptr, vt[h * 64:(h + 1) * 64, ts(j, 128)],
                            id_sb[h * 64:(h + 1) * 64, :],
                        )
                        nc.vector.tensor_copy(
                            vsb[:, j * 130 + h * 65: j * 130 + h * 65 + 64], ptr
                        )

                # ================= attention for batch b ===================
                # h1's normalized rows are produced at base partition 0 (DVE
                # lanes are partition-locked) and DMA'd to partitions 64..127
                # of onorm at the end of the batch.
                onorm = seq.tile([128, T], mdt, tag="onorm")
                onorm1 = seq.tile([64, T], mdt, tag="onorm1")
                for p in range(NPAIR):
                    nj = p * (PW // 128) + (PW // 128)       # j in [0, nj)
                    acc = [[pacc.tile([65, CH], f32, tag="acc", name="acc")
                            for _ in range(PW // CH)] for _ in (0, 1)]
                    for j in range(nj):
                        j_rel = j - p * (PW // 128)
                        c0 = max(0, 128 * j_rel)             # first valid col
                        qq_lo = c0 // CH                     # first chunk kept
                        for h in (0, 1):
                            S = pmm.tile([128, PW], f32, tag="s")
                            col = c0
                            while col < PW:
                                hi = min(PW, (col // CH + 1) * CH)
                                nc.tensor.matmul(
                                    S[:, col:hi],
                                    rc(kt[h * 64:(h + 1) * 64, ts(j, 128)]),
                                    rc(qt[h * 64:(h + 1) * 64,
                                          p * PW + col: p * PW + hi]),
                                    start=True, stop=True,
                                )
                                col = hi
                            if j_rel >= 0:       # diagonal: mask on PSUM
                                nc.vector.tensor_add(
                                    S[:, c0:c0 + 128], S[:, c0:c0 + 128],
                                    tri_sb,
                                )
                            E = att.tile([128, PW], mdt, tag="expt")
                            nc.scalar.activation(
                                out=E[:, c0:PW], in_=S[:, c0:PW],
                                func=mybir.ActivationFunctionType.Exp,
                                scale=SCALE,
                            )
                            for qq in range(qq_lo, PW // CH):
                                last = nj - 1 if qq > 0 else \
                                    min(nj - 1, p * (PW // 128) + 3)
                                lo = max(c0, qq * CH)
                                nc.tensor.matmul(
                                    acc[h][qq][:, lo - qq * CH: CH],
                                    rc(vsb[:, j * 130 + h * 65:
                                           j * 130 + h * 65 + 65]),
                                    rc(E[:, lo:(qq + 1) * CH]),
                                    start=(j == 0), stop=(j == last),
                                    skip_group_check=True,
                                )
                    # normalize: O' rows 0..63 per head, denominator row 64.
                    # d-row -> SBUF (aligned copy at partition 64), broadcast
                    # to 64 partitions with a K=1 matmul, reciprocal, then
                    # scale O' straight out of PSUM.
                    for qq in range(PW // CH):
                        for h in (0, 1):
                            dsb = small.tile([65, CH], ddt, tag="dsb",
                                             name="dsb")
                            nc.vector.tensor_copy(
                                dsb[64:65, :], acc[h][qq][64:65, :]
                            )
                            dbc = pmm.tile([64, CH], f32, tag="s", name="dbc")
                            nc.tensor.matmul(
                                dbc,
                                rc(ones2_sb[64:65, :]),
                                rc(dsb[64:65, :]),
                                start=True, stop=True,
                            )
                            rcp = small.tile([64, CH], f32, tag="rcp",
                                             name="rcp")
                            nc.vector.reciprocal(rcp, dbc)
                            dst = onorm if h == 0 else onorm1
                            nc.vector.tensor_mul(
                                dst[0:64,
                                    p * PW + qq * CH: p * PW + (qq + 1) * CH],
                                acc[h][qq][0:64, :],
                                rcp,
                            )
                # place h1 rows at partitions 64..127 (DMA moves across
                # partitions; DVE cannot)
                nc.sync.dma_start(out=onorm[64:128, :], in_=onorm1)

                # ================= phase C: partial W_o for batch b ========
                for tc_i in range(NT):
                    osb = stream.tile([128, DM], f32, tag="osb")
                    for cc in range(DM // CH):
                        po = pmm.tile([128, CH], f32, tag="s")
                        nc.tensor.matmul(
                            po,
                            rc(onorm[:, ts(tc_i, 128)]),
                            rc(wo_sb[:, ts(cc, CH)]),
                            start=True, stop=True,
                        )
                        nc.vector.tensor_add(
                            osb[:, ts(cc, CH)], po, bo_sb[:, ts(cc, CH)]
                        )
                    r0 = b * T + tc_i * 128
                    nc.sync.dma_start(out=outp[r0:r0 + 128, :], in_=osb)

    nc.compile()
    return nc


MODE = "f32r"


def _get_nc():
    key = "nc" + MODE
    if key not in _CACHE:
        _CACHE[key] = _build(MODE)
    return _CACHE[key]


def make_in_maps(x, Wq, Wk, Wv, Wo, bo):
    x2d = np.ascontiguousarray(x.reshape(BT, DM), dtype=np.float32)
    xT = np.ascontiguousarray(x2d.T)
    bo8 = np.ascontiguousarray((bo / NCORES).reshape(1, DM), dtype=np.float32)
    maps = []
    for c in range(NCORES):
        h0, h1 = HL * c, HL * c + 1
        maps.append({
            "xT": xT,
            "wq2": np.ascontiguousarray(
                np.concatenate([Wq[h0], Wq[h1]], 1), dtype=np.float32),
            "wk2": np.ascontiguousarray(
                np.concatenate([Wk[h0], Wk[h1]], 1), dtype=np.float32),
            "wv2": np.ascontiguousarray(
                np.concatenate([Wv[h0], Wv[h1]], 1), dtype=np.float32),
            "wo_my": np.ascontiguousarray(
                Wo[SD * c: SD * (c + 1)], dtype=np.float32),
            "bo8": bo8,
        })
    return maps


def run(x, Wq, Wk, Wv, Wo, bo, trace=False, **spmd_kwargs):
    from concourse.bass_utils import run_bass_kernel_spmd

    nc = _get_nc()
    maps = make_in_maps(x, Wq, Wk, Wv, Wo, bo)
    res = run_bass_kernel_spmd(
        nc, maps, core_ids=list(range(NCORES)), trace=trace, **spmd_kwargs
    )
    total = np.zeros((BT, DM), np.float32)
    for r in res.results:
        total += r["out_part"]
    return total.reshape(B, T, DM), res


def kernel(x, Wq, Wk, Wv, Wo, bo):
    out, _ = run(x, Wq, Wk, Wv, Wo, bo)
    return out
